# revision 15
# baseline (speedup 1.0000x reference)
"""DressedQuantumNet on 8 TRN2 NeuronCores (pure data parallel).

Math: pre-net angles th = X @ pre_w.T (+ pre_b).  After H + RY(th') the
4-qubit state is the REAL product state

  psi = kron_w [cos(th'_w/2 + pi/4), sin(th'_w/2 + pi/4)],  th' = th + pre_b

and the rest of the circuit is a FIXED unitary V (depends only on
q_weights).  The per-wire bias rotation R(pre_b_w/2) is absorbed into
V' = V @ kron_w R(pre_b_w/2), so the device only needs raw th.  With
Wz[i,c] = sum_w post_w[c,w] z_w(i) + post_b[c] (sum_i probs_i == 1), the
whole head collapses to two real symmetric quadratic forms

  out_c = psi^T K_c psi,   K_c = Re(V'^H diag(Wz[:,c]) V')   [16 x 16]

Device pipeline per 8-row-tile slab (batch on SBUF partitions), software
pipelined A(s) | B(s-1) | C(s-2) to keep all engine FIFOs unblocked:

  A: DMA fp16 X^T slab (1 MiB, sync queue; consts preloaded on same queue)
     PE  4 accumulating matmuls/tile -> th in PSUM [128,t,4]
     ACT cs = Sin(+-0.5*th + pi/4)  -> (cos,sin) fp16
     DVE psi = (c0,s0)x(c1,s1)x(c2,s2)x(c3,s3)  [128, t, 16] fp16
  B: PE  transpose psi -> psiT [16t, 128] PSUM; ACT copy -> SBUF
     PE  qq_c = blockdiag(K_c) @ psiT  (c=0,1 -> one PSUM tile)
  C: DVE pq_c = psiT * qq_c  (SBUF x PSUM -> fp16)
     PE  z[t, c, p] = column-sum over the 16 states (selection matmul)
     ACT copy z -> resall; one DMA out at the end

Everything sits under the fp16 input-stream DMA floor (~8.4 MiB/core).
"""

from contextlib import ExitStack

import numpy as np

import concourse.bass as bass
import concourse.bacc as bacc_mod
import concourse.mybir as mybir
from concourse.bass_utils import run_bass_kernel_spmd
from concourse.tile import TileContext

N_CORES = 8
B_TOTAL = 65536
F_IN = 512
ROWS = B_TOTAL // N_CORES   # 8192 rows per core
P = 128
N_TILES = ROWS // P         # 64 row-tiles
G = 8                       # row-tiles per slab (1 MiB fp16 DMA)
N_SLABS = N_TILES // G      # 8

F32 = mybir.dt.float32
FP16 = mybir.dt.float16
PI = float(np.pi)

N_QUBITS, VAR_DEPTH = 4, 3


# ----------------------------------------------------------------- host math
def _gate_1q(g, w):
    ops = [np.eye(2, dtype=complex)] * N_QUBITS
    ops[w] = g
    U = ops[0]
    for i in range(1, N_QUBITS):
        U = np.kron(U, ops[i])
    return U


def _bit(i, w):  # wire 0 = most significant
    return (i >> (N_QUBITS - 1 - w)) & 1


def _cnot(c, t):
    M = np.zeros((16, 16), dtype=complex)
    for i in range(16):
        j = i ^ (1 << (N_QUBITS - 1 - t)) if _bit(i, c) else i
        M[j, i] = 1.0
    return M


def _ry(theta):
    c, s = np.cos(theta / 2), np.sin(theta / 2)
    return np.array([[c, -s], [s, c]], dtype=complex)


def _rz(theta):
    ph = np.exp(1j * theta / 2)
    return np.array([[np.conj(ph), 0], [0, ph]], dtype=complex)


def _fixed_unitary(qw):
    V = np.eye(16, dtype=complex)

    def app(Gm):
        nonlocal V
        V = Gm @ V

    def entangle():
        app(_cnot(0, 1)); app(_cnot(2, 3)); app(_cnot(1, 2))

    for k in range(VAR_DEPTH):
        entangle()
        for w in range(N_QUBITS):
            app(_gate_1q(_ry(qw[k, w]), w))
        for w in range(N_QUBITS):
            app(_gate_1q(_rz(qw[k, w]), w))
    for k in range(VAR_DEPTH):
        entangle()
        for w in range(N_QUBITS):
            app(_gate_1q(_ry(qw[k, w]), w))
        for w in range(N_QUBITS):
            app(_gate_1q(_rz(qw[3 + k, w]), w))
    entangle()
    return V


def _host_consts(pre_w, pre_b, q_weights, post_w, post_b):
    pre_w = np.asarray(pre_w, dtype=np.float64)
    pre_b = np.asarray(pre_b, dtype=np.float64)
    post_w = np.asarray(post_w, dtype=np.float64)
    post_b = np.asarray(post_b, dtype=np.float64)

    # whl[p, 4k + w] = pre_w[w, 128k + p]
    whl = np.zeros((P, 16), dtype=np.float16)
    for k in range(4):
        whl[:, 4 * k:4 * k + 4] = pre_w.T[P * k:P * (k + 1)].astype(np.float16)

    V = _fixed_unitary(np.asarray(q_weights, dtype=np.float64))
    R = np.eye(1)
    for w in range(N_QUBITS):
        d = pre_b[w] / 2.0
        R = np.kron(R, np.array([[np.cos(d), -np.sin(d)],
                                 [np.sin(d), np.cos(d)]]))
    Vp = V @ R

    # Wz[i, c] = sum_w post_w[c,w] z_w(i) + post_b[c]  (sum_i probs_i == 1)
    Wz = np.zeros((16, 2))
    for c in range(2):
        for i in range(16):
            Wz[i, c] = sum(
                post_w[c, w] * (1.0 - 2.0 * _bit(i, w)) for w in range(N_QUBITS)
            ) + post_b[c]

    # K_c = Re(V'^H diag(Wz_c) V')  -- real symmetric 16x16; block-diagonal
    kb = []
    for c in range(2):
        Kc = (Vp.conj().T @ np.diag(Wz[:, c]) @ Vp).real
        blk = np.zeros((P, P), dtype=np.float16)
        for t in range(G):
            blk[16 * t:16 * t + 16, 16 * t:16 * t + 16] = Kc.T.astype(np.float16)
        kb.append(blk)

    selz = np.zeros((P, G), dtype=np.float16)
    for t in range(G):
        selz[16 * t:16 * t + 16, t] = 1.0

    ident = np.eye(P, dtype=np.float16)
    blob = np.concatenate([whl, kb[0], kb[1], selz, ident], axis=1)
    return {"cblob": np.ascontiguousarray(blob)}


# ------------------------------------------------------------- device kernel
SCHED = [4, 4] + [8] * 6 + [4, 4]   # tiles per DMA slab (sum = 64)
UNITS = 8                           # 8-tile epilogue units
BATCHES = [4, 2, 1, 1]              # units per epilogue batch (sum = 8)


def build_bass(rows=ROWS):
    n_tiles = rows // P
    assert sum(SCHED) == n_tiles
    n_units = n_tiles // G

    nc = bacc_mod.Bacc(None, target_bir_lowering=False)
    # host-packed flat: concatenation of per-slab [P, 4, g*P] fp16 blocks
    ht_d = nc.dram_tensor("htp", [rows * 4 * P], FP16, kind="ExternalInput")
    cb_d = nc.dram_tensor("cblob", [P, 16 + 3 * P + G], FP16, kind="ExternalInput")
    # out_dev[t, c, u, p] = out[(u*G + t)*128 + p, c]; host unscrambles
    out_d = nc.dram_tensor("out", [G, 2, n_units, P], F32, kind="ExternalOutput")

    with TileContext(nc) as tc, ExitStack() as ctx:
        const = ctx.enter_context(tc.tile_pool(name="const", bufs=1))
        cblob = const.tile([P, 16 + 3 * P + G], FP16)
        nc.sync.dma_start(cblob, cb_d[:])
        whl = cblob[:, 0:16]
        k0b = cblob[:, 16:16 + P]
        k1b = cblob[:, 16 + P:16 + 2 * P]
        selz = cblob[:, 16 + 2 * P:16 + 2 * P + G]
        ident = cblob[:, 16 + 2 * P + G:16 + 3 * P + G]
        pi4 = const.tile([P, 1], F32)
        nc.vector.memset(pi4, PI / 4)

        xp = ctx.enter_context(tc.tile_pool(name="xin", bufs=len(SCHED)))
        angp = ctx.enter_context(tc.tile_pool(name="angp", bufs=3, space="PSUM"))
        csp = ctx.enter_context(tc.tile_pool(name="csp", bufs=3))
        pp = ctx.enter_context(tc.tile_pool(name="pp", bufs=3))
        psip = ctx.enter_context(tc.tile_pool(name="psip", bufs=6))
        ptp = ctx.enter_context(tc.tile_pool(name="ptp", bufs=1, space="PSUM"))
        pts = ctx.enter_context(tc.tile_pool(name="pts", bufs=2))
        qqp = ctx.enter_context(tc.tile_pool(name="qqp", bufs=1, space="PSUM"))
        prp = ctx.enter_context(tc.tile_pool(name="prp", bufs=2))
        zp = ctx.enter_context(tc.tile_pool(name="zp", bufs=1, space="PSUM"))
        rp = ctx.enter_context(tc.tile_pool(name="res", bufs=1))

        resall = rp.tile([G, 2, n_units, P], F32)

        def stage_a(s, g, base):
            gb = g * P
            ht = xp.tile([P, 4, gb], FP16, tag="ht")
            nc.sync.dma_start(
                ht,
                ht_d[base:base + P * 4 * gb].rearrange(
                    "(p k b) -> p k b", p=P, k=4),
            )
            # th[p, t, w] in PSUM, fp32
            ang = angp.tile([P, g, 4], F32, tag="ang")
            for t in range(g):
                bs = t * P
                for k in range(4):
                    nc.tensor.matmul(
                        ang[:, t, :],
                        ht[:, k, bs:bs + P],
                        whl[:, 4 * k:4 * k + 4],
                        start=(k == 0), stop=(k == 3),
                    )
            # cs[p, t, w, 0] = cos(th/2 + pi/4) = Sin(-.5*th + pi/4)
            # cs[p, t, w, 1] = sin(th/2 + pi/4) = Sin(+.5*th + pi/4)
            cs = csp.tile([P, g, 4, 2], FP16, tag="cs")
            nc.scalar.activation(
                cs[:, :, :, 0], ang, mybir.ActivationFunctionType.Sin,
                bias=pi4, scale=-0.5,
            )
            nc.scalar.activation(
                cs[:, :, :, 1], ang, mybir.ActivationFunctionType.Sin,
                bias=pi4, scale=0.5,
            )
            # psi = kron of the four (c,s) pairs -> [P, g, 4, 4] fp16
            p01 = pp.tile([P, g, 2, 2], FP16, tag="p01")
            nc.vector.tensor_mul(
                p01,
                cs[:, :, 0, :].unsqueeze(3).broadcast_to([P, g, 2, 2]),
                cs[:, :, 1, :].unsqueeze(2).broadcast_to([P, g, 2, 2]),
            )
            p23 = pp.tile([P, g, 2, 2], FP16, tag="p23")
            nc.vector.tensor_mul(
                p23,
                cs[:, :, 2, :].unsqueeze(3).broadcast_to([P, g, 2, 2]),
                cs[:, :, 3, :].unsqueeze(2).broadcast_to([P, g, 2, 2]),
            )
            psi = psip.tile([P, g, 4, 4], FP16, tag="psi")
            nc.vector.tensor_mul(
                psi,
                p01.rearrange("p g a b -> p g (a b)")
                   .unsqueeze(3).broadcast_to([P, g, 4, 4]),
                p23.rearrange("p g a b -> p g (a b)")
                   .unsqueeze(2).broadcast_to([P, g, 4, 4]),
            )
            return psi

        def epilogue(b, nu, u0):
            """Units u0..u0+nu-1: psiT already in ptile; contract and emit z."""
            ptile, _ = ptiles[b]
            psiTs = pts.tile([P, nu, P], FP16, tag="psiTs")
            nc.scalar.copy(psiTs, ptile)
            rhs = psiTs.rearrange("p u b -> p (u b)")
            qq0 = qqp.tile([P, nu, P], F32, tag="qq0")
            nc.tensor.matmul(qq0.rearrange("p u b -> p (u b)"), k0b, rhs,
                             start=True, stop=True)
            qq1 = qqp.tile([P, nu, P], F32, tag="qq1")
            nc.tensor.matmul(qq1.rearrange("p u b -> p (u b)"), k1b, rhs,
                             start=True, stop=True)
            pq = prp.tile([P, 2, nu, P], FP16, tag="pq")
            nc.vector.tensor_mul(pq[:, 0], qq0, psiTs)
            nc.vector.tensor_mul(pq[:, 1], qq1, psiTs)
            for c, qt in ((0, "z0"), (1, "z1")):
                z_ps = zp.tile([G, nu, P], F32, tag=qt)
                nc.tensor.matmul(
                    z_ps.rearrange("t u b -> t (u b)"), selz,
                    pq[:, c].rearrange("p u b -> p (u b)"),
                    start=True, stop=True,
                )
                nc.scalar.copy(resall[:, c, u0:u0 + nu, :], z_ps)
            nc.gpsimd.dma_start(out_d[:, :, u0:u0 + nu, :],
                                resall[:, :, u0:u0 + nu, :])

        # batch layout: which (slab -> batch, unit-in-batch, tile-offset)
        slab_info = []          # (s, g, dram_base, batch, ucol, toff)
        batch_of_unit, ucol_of_unit, u0_of_batch = {}, {}, {}
        u = 0
        for b, nu in enumerate(BATCHES):
            u0_of_batch[b] = u
            for k in range(nu):
                batch_of_unit[u] = b
                ucol_of_unit[u] = k
                u += 1
        tile0, base = 0, 0
        for si, g in enumerate(SCHED):
            unit, toff = tile0 // G, tile0 % G
            assert toff + g <= G, "slab may not straddle units"
            slab_info.append(
                (si, g, base, batch_of_unit[unit], ucol_of_unit[unit], toff))
            tile0 += g
            base += P * 4 * g * P
        last_slab_of_batch = {}
        for si, g, base, b, ucol, toff in slab_info:
            last_slab_of_batch[b] = si

        ptiles = {}
        pending = []            # (slab, psi) awaiting transpose
        done_batches = []

        def transpose_slab(si):
            s, g, base, b, ucol, toff = slab_info[si]
            if b not in ptiles:
                nu = BATCHES[b]
                ptiles[b] = (ptp.tile([P, nu, P], FP16, tag="pt", name="ptile"), nu)
            ptile, _ = ptiles[b]
            psi = psis.pop(si)
            nc.tensor.transpose(
                ptile[16 * toff:16 * (toff + len_g[si]), ucol, :],
                psi.rearrange("p g a b -> p (g a b)"), ident,
            )

        psis = {}
        len_g = {si: g for si, g, *_ in slab_info}
        n_slabs = len(SCHED)
        TL = 3                      # transpose emission lag (slabs)
        epi_q = []
        for s in range(n_slabs + TL):
            while epi_q:
                b = epi_q.pop(0)
                epilogue(b, BATCHES[b], u0_of_batch[b])
            if s < n_slabs:
                si, g, base, b, ucol, toff = slab_info[s]
                psis[s] = stage_a(si, g, base)
            if s >= TL:
                st = s - TL
                transpose_slab(st)
                bprev = slab_info[st][3]
                if last_slab_of_batch[bprev] == st:
                    epi_q.append(bprev)
        while epi_q:
            b = epi_q.pop(0)
            epilogue(b, BATCHES[b], u0_of_batch[b])

    nc.finalize()
    return nc


_NC_CACHE = {}


def _get_nc(rows=ROWS):
    if rows not in _NC_CACHE:
        _NC_CACHE[rows] = build_bass(rows=rows)
    return _NC_CACHE[rows]


def _pack_input(x):
    """x [ROWS, F] f32 -> flat fp16: per-slab [P, 4, g*P] packs,
    pack[p, k, b] = x[slab_row0 + b, 128*k + p]."""
    h = x.astype(np.float16)
    parts = []
    r0 = 0
    for g in SCHED:
        gb = g * P
        blk = h[r0:r0 + gb].reshape(gb, 4, P).transpose(2, 1, 0)
        parts.append(np.ascontiguousarray(blk).reshape(-1))
        r0 += gb
    return np.concatenate(parts)


def run(input_features, pre_w, pre_b, q_weights, post_w, post_b, **spmd_kwargs):
    x = np.asarray(input_features, dtype=np.float32)
    assert x.shape == (B_TOTAL, F_IN), x.shape
    consts = _host_consts(pre_w, pre_b, q_weights, post_w, post_b)
    in_maps = []
    for c in range(N_CORES):
        ht = _pack_input(x[c * ROWS:(c + 1) * ROWS])
        in_maps.append(dict(consts, htp=ht))
    nc = _get_nc()
    r = run_bass_kernel_spmd(nc, in_maps, core_ids=list(range(N_CORES)), **spmd_kwargs)
    # out_dev[t, c, s, p] -> out[(s*G + t)*128 + p, c]
    outs = []
    for c in range(N_CORES):
        o = r.results[c]["out"]                             # [t, c, s, p]
        o = o.transpose(2, 0, 3, 1).reshape(ROWS, 2)        # [s, t, p, c]
        outs.append(o)
    out = np.concatenate(outs, axis=0)
    return out.astype(np.float32), r


def kernel(input_features, pre_w, pre_b, q_weights, post_w, post_b):
    out, _ = run(input_features, pre_w, pre_b, q_weights, post_w, post_b)
    return out


# revision 16
# speedup vs baseline: 1.0211x; 1.0211x over previous
"""DressedQuantumNet on 8 TRN2 NeuronCores (pure data parallel).

Math: pre-net angles th = X @ pre_w.T (+ pre_b).  After H + RY(th') the
4-qubit state is the REAL product state

  psi = kron_w [cos(th'_w/2 + pi/4), sin(th'_w/2 + pi/4)],  th' = th + pre_b

and the rest of the circuit is a FIXED unitary V (depends only on
q_weights).  The per-wire bias rotation R(pre_b_w/2) is absorbed into
V' = V @ kron_w R(pre_b_w/2), so the device only needs raw th.  With
Wz[i,c] = sum_w post_w[c,w] z_w(i) + post_b[c] (sum_i probs_i == 1), the
whole head collapses to two real symmetric quadratic forms

  out_c = psi^T K_c psi,   K_c = Re(V'^H diag(Wz[:,c]) V')   [16 x 16]

Device pipeline per 8-row-tile slab (batch on SBUF partitions), software
pipelined A(s) | B(s-1) | C(s-2) to keep all engine FIFOs unblocked:

  A: DMA fp16 X^T slab (1 MiB, sync queue; consts preloaded on same queue)
     PE  4 accumulating matmuls/tile -> th in PSUM [128,t,4]
     ACT cs = Sin(+-0.5*th + pi/4)  -> (cos,sin) fp16
     DVE psi = (c0,s0)x(c1,s1)x(c2,s2)x(c3,s3)  [128, t, 16] fp16
  B: PE  transpose psi -> psiT [16t, 128] PSUM; ACT copy -> SBUF
     PE  qq_c = blockdiag(K_c) @ psiT  (c=0,1 -> one PSUM tile)
  C: DVE pq_c = psiT * qq_c  (SBUF x PSUM -> fp16)
     PE  z[t, c, p] = column-sum over the 16 states (selection matmul)
     ACT copy z -> resall; one DMA out at the end

Everything sits under the fp16 input-stream DMA floor (~8.4 MiB/core).
"""

from contextlib import ExitStack

import numpy as np

import concourse.bass as bass
import concourse.bacc as bacc_mod
import concourse.mybir as mybir
from concourse.bass_utils import run_bass_kernel_spmd
from concourse.tile import TileContext

N_CORES = 8
B_TOTAL = 65536
F_IN = 512
ROWS = B_TOTAL // N_CORES   # 8192 rows per core
P = 128
N_TILES = ROWS // P         # 64 row-tiles
G = 8                       # row-tiles per slab (1 MiB fp16 DMA)
N_SLABS = N_TILES // G      # 8

F32 = mybir.dt.float32
FP16 = mybir.dt.float16
PI = float(np.pi)

N_QUBITS, VAR_DEPTH = 4, 3


# ----------------------------------------------------------------- host math
def _gate_1q(g, w):
    ops = [np.eye(2, dtype=complex)] * N_QUBITS
    ops[w] = g
    U = ops[0]
    for i in range(1, N_QUBITS):
        U = np.kron(U, ops[i])
    return U


def _bit(i, w):  # wire 0 = most significant
    return (i >> (N_QUBITS - 1 - w)) & 1


def _cnot(c, t):
    M = np.zeros((16, 16), dtype=complex)
    for i in range(16):
        j = i ^ (1 << (N_QUBITS - 1 - t)) if _bit(i, c) else i
        M[j, i] = 1.0
    return M


def _ry(theta):
    c, s = np.cos(theta / 2), np.sin(theta / 2)
    return np.array([[c, -s], [s, c]], dtype=complex)


def _rz(theta):
    ph = np.exp(1j * theta / 2)
    return np.array([[np.conj(ph), 0], [0, ph]], dtype=complex)


def _fixed_unitary(qw):
    V = np.eye(16, dtype=complex)

    def app(Gm):
        nonlocal V
        V = Gm @ V

    def entangle():
        app(_cnot(0, 1)); app(_cnot(2, 3)); app(_cnot(1, 2))

    for k in range(VAR_DEPTH):
        entangle()
        for w in range(N_QUBITS):
            app(_gate_1q(_ry(qw[k, w]), w))
        for w in range(N_QUBITS):
            app(_gate_1q(_rz(qw[k, w]), w))
    for k in range(VAR_DEPTH):
        entangle()
        for w in range(N_QUBITS):
            app(_gate_1q(_ry(qw[k, w]), w))
        for w in range(N_QUBITS):
            app(_gate_1q(_rz(qw[3 + k, w]), w))
    entangle()
    return V


def _host_consts(pre_w, pre_b, q_weights, post_w, post_b):
    pre_w = np.asarray(pre_w, dtype=np.float64)
    pre_b = np.asarray(pre_b, dtype=np.float64)
    post_w = np.asarray(post_w, dtype=np.float64)
    post_b = np.asarray(post_b, dtype=np.float64)

    # whl[p, 4k + w] = pre_w[w, 128k + p]
    whl = np.zeros((P, 16), dtype=np.float16)
    for k in range(4):
        whl[:, 4 * k:4 * k + 4] = pre_w.T[P * k:P * (k + 1)].astype(np.float16)

    V = _fixed_unitary(np.asarray(q_weights, dtype=np.float64))
    R = np.eye(1)
    for w in range(N_QUBITS):
        d = pre_b[w] / 2.0
        R = np.kron(R, np.array([[np.cos(d), -np.sin(d)],
                                 [np.sin(d), np.cos(d)]]))
    Vp = V @ R

    # Wz[i, c] = sum_w post_w[c,w] z_w(i) + post_b[c]  (sum_i probs_i == 1)
    Wz = np.zeros((16, 2))
    for c in range(2):
        for i in range(16):
            Wz[i, c] = sum(
                post_w[c, w] * (1.0 - 2.0 * _bit(i, w)) for w in range(N_QUBITS)
            ) + post_b[c]

    # K_c = Re(V'^H diag(Wz_c) V')  -- real symmetric 16x16; block-diagonal
    kb = []
    for c in range(2):
        Kc = (Vp.conj().T @ np.diag(Wz[:, c]) @ Vp).real
        blk = np.zeros((P, P), dtype=np.float16)
        for t in range(G):
            blk[16 * t:16 * t + 16, 16 * t:16 * t + 16] = Kc.T.astype(np.float16)
        kb.append(blk)

    selz = np.zeros((P, G), dtype=np.float16)
    for t in range(G):
        selz[16 * t:16 * t + 16, t] = 1.0

    ident = np.eye(P, dtype=np.float16)
    blob = np.concatenate([whl, kb[0], kb[1], selz, ident], axis=1)
    return {"cblob": np.ascontiguousarray(blob)}


# ------------------------------------------------------------- device kernel
SCHED = [8, 16, 16, 16, 4, 4]       # tiles per DMA slab (sum = 64)
UNITS = 8                           # 8-tile epilogue units
BATCHES = [4, 2, 2]                 # units per epilogue batch (sum = 8)


def build_bass(rows=ROWS):
    n_tiles = rows // P
    assert sum(SCHED) == n_tiles
    n_units = n_tiles // G

    nc = bacc_mod.Bacc(None, target_bir_lowering=False)
    # host-packed flat: concatenation of per-slab [P, 4, g*P] fp16 blocks
    ht_d = nc.dram_tensor("htp", [rows * 4 * P], FP16, kind="ExternalInput")
    cb_d = nc.dram_tensor("cblob", [P, 16 + 3 * P + G], FP16, kind="ExternalInput")
    # out_dev[t, c, u, p] = out[(u*G + t)*128 + p, c]; host unscrambles
    out_d = nc.dram_tensor("out", [G, 2, n_units, P], F32, kind="ExternalOutput")

    with TileContext(nc) as tc, ExitStack() as ctx:
        const = ctx.enter_context(tc.tile_pool(name="const", bufs=1))
        cblob = const.tile([P, 16 + 3 * P + G], FP16)
        nc.sync.dma_start(cblob, cb_d[:])
        whl = cblob[:, 0:16]
        k0b = cblob[:, 16:16 + P]
        k1b = cblob[:, 16 + P:16 + 2 * P]
        selz = cblob[:, 16 + 2 * P:16 + 2 * P + G]
        ident = cblob[:, 16 + 2 * P + G:16 + 3 * P + G]
        pi4 = const.tile([P, 1], F32)
        nc.vector.memset(pi4, PI / 4)

        xp = ctx.enter_context(tc.tile_pool(name="xin", bufs=len(SCHED)))
        angp = ctx.enter_context(tc.tile_pool(name="angp", bufs=2, space="PSUM"))
        csp = ctx.enter_context(tc.tile_pool(name="csp", bufs=3))
        pp = ctx.enter_context(tc.tile_pool(name="pp", bufs=3))
        psip = ctx.enter_context(tc.tile_pool(name="psip", bufs=6))
        ptp = ctx.enter_context(tc.tile_pool(name="ptp", bufs=1, space="PSUM"))
        pts = ctx.enter_context(tc.tile_pool(name="pts", bufs=2))
        qqp = ctx.enter_context(tc.tile_pool(name="qqp", bufs=1, space="PSUM"))
        prp = ctx.enter_context(tc.tile_pool(name="prp", bufs=2))
        zp = ctx.enter_context(tc.tile_pool(name="zp", bufs=1, space="PSUM"))
        rp = ctx.enter_context(tc.tile_pool(name="res", bufs=1))

        resall = rp.tile([G, 2, n_units, P], F32)

        def stage_a(s, g, base):
            gb = g * P
            ht = xp.tile([P, 4, gb], FP16, tag="ht")
            nc.sync.dma_start(
                ht,
                ht_d[base:base + P * 4 * gb].rearrange(
                    "(p k b) -> p k b", p=P, k=4),
            )
            # th[p, t, w] in PSUM, fp32
            ang = angp.tile([P, g, 4], F32, tag="ang")
            for t in range(g):
                bs = t * P
                for k in range(4):
                    nc.tensor.matmul(
                        ang[:, t, :],
                        ht[:, k, bs:bs + P],
                        whl[:, 4 * k:4 * k + 4],
                        start=(k == 0), stop=(k == 3),
                    )
            # cs[p, t, w, 0] = cos(th/2 + pi/4) = Sin(-.5*th + pi/4)
            # cs[p, t, w, 1] = sin(th/2 + pi/4) = Sin(+.5*th + pi/4)
            cs = csp.tile([P, g, 4, 2], FP16, tag="cs")
            nc.scalar.activation(
                cs[:, :, :, 0], ang, mybir.ActivationFunctionType.Sin,
                bias=pi4, scale=-0.5,
            )
            nc.scalar.activation(
                cs[:, :, :, 1], ang, mybir.ActivationFunctionType.Sin,
                bias=pi4, scale=0.5,
            )
            # psi = kron of the four (c,s) pairs -> [P, g, 4, 4] fp16
            p01 = pp.tile([P, g, 2, 2], FP16, tag="p01")
            nc.vector.tensor_mul(
                p01,
                cs[:, :, 0, :].unsqueeze(3).broadcast_to([P, g, 2, 2]),
                cs[:, :, 1, :].unsqueeze(2).broadcast_to([P, g, 2, 2]),
            )
            p23 = pp.tile([P, g, 2, 2], FP16, tag="p23")
            nc.vector.tensor_mul(
                p23,
                cs[:, :, 2, :].unsqueeze(3).broadcast_to([P, g, 2, 2]),
                cs[:, :, 3, :].unsqueeze(2).broadcast_to([P, g, 2, 2]),
            )
            psi = psip.tile([P, g, 4, 4], FP16, tag="psi")
            nc.vector.tensor_mul(
                psi,
                p01.rearrange("p g a b -> p g (a b)")
                   .unsqueeze(3).broadcast_to([P, g, 4, 4]),
                p23.rearrange("p g a b -> p g (a b)")
                   .unsqueeze(2).broadcast_to([P, g, 4, 4]),
            )
            return psi

        def epilogue(b, nu, u0):
            """Units u0..u0+nu-1: psiT already in ptile; contract and emit z."""
            ptile = ptiles.pop(b)
            psiTs = pts.tile([P, nu, P], FP16, tag="psiTs")
            nc.scalar.copy(psiTs, ptile)
            rhs = psiTs.rearrange("p u b -> p (u b)")
            qq0 = qqp.tile([P, nu, P], F32, tag="qq0")
            nc.tensor.matmul(qq0.rearrange("p u b -> p (u b)"), k0b, rhs,
                             start=True, stop=True)
            qq1 = qqp.tile([P, nu, P], F32, tag="qq1")
            nc.tensor.matmul(qq1.rearrange("p u b -> p (u b)"), k1b, rhs,
                             start=True, stop=True)
            pq = prp.tile([P, 2, nu, P], FP16, tag="pq")
            nc.vector.tensor_mul(pq[:, 0], qq0, psiTs)
            nc.vector.tensor_mul(pq[:, 1], qq1, psiTs)
            for c, qt in ((0, "z0"), (1, "z1")):
                z_ps = zp.tile([G, nu, P], F32, tag=qt)
                nc.tensor.matmul(
                    z_ps.rearrange("t u b -> t (u b)"), selz,
                    pq[:, c].rearrange("p u b -> p (u b)"),
                    start=True, stop=True,
                )
                nc.scalar.copy(resall[:, c, u0:u0 + nu, :], z_ps)
            nc.gpsimd.dma_start(out_d[:, :, u0:u0 + nu, :],
                                resall[:, :, u0:u0 + nu, :])

        # batch/unit layout
        batch_of_unit, ucol_of_unit, u0_of_batch = {}, {}, {}
        u = 0
        for b, nu in enumerate(BATCHES):
            u0_of_batch[b] = u
            for k in range(nu):
                batch_of_unit[u] = b
                ucol_of_unit[u] = k
                u += 1
        n_units_total = u
        # per-slab: dram base + unit chunks (unit, toff, chunk_start, chunk_len)
        slab_info = []
        tile0, base = 0, 0
        for si, g in enumerate(SCHED):
            chunks = []
            t = tile0
            while t < tile0 + g:
                unit, toff = t // G, t % G
                ln = min(G - toff, tile0 + g - t)
                chunks.append((unit, toff, t - tile0, ln))
                t += ln
            slab_info.append((g, base, chunks))
            tile0 += g
            base += P * 4 * g * P
        units_left = {b: nu for b, nu in enumerate(BATCHES)}

        ptiles = {}

        def transpose_slab(si):
            g, base, chunks = slab_info[si]
            psi = psis.pop(si)
            done_batches = []
            for unit, toff, cs0, ln in chunks:
                b = batch_of_unit[unit]
                if b not in ptiles:
                    ptiles[b] = ptp.tile([P, BATCHES[b], P], FP16, tag="pt",
                                         name="ptile")
                nc.tensor.transpose(
                    ptiles[b][16 * toff:16 * (toff + ln), ucol_of_unit[unit], :],
                    psi[:, cs0:cs0 + ln, :, :].rearrange("p g a b -> p (g a b)"),
                    ident,
                )
                if toff + ln == G:
                    units_left[b] -= 1
                    if units_left[b] == 0:
                        done_batches.append(b)
            return done_batches

        psis = {}
        n_slabs = len(SCHED)
        TL = 1                      # transpose emission lag (slabs)
        epi_q = []
        for s in range(n_slabs + TL):
            while epi_q:
                b = epi_q.pop(0)
                epilogue(b, BATCHES[b], u0_of_batch[b])
            if s < n_slabs:
                g, base, chunks = slab_info[s]
                psis[s] = stage_a(s, g, base)
            if s >= TL:
                epi_q.extend(transpose_slab(s - TL))
        while epi_q:
            b = epi_q.pop(0)
            epilogue(b, BATCHES[b], u0_of_batch[b])

    nc.finalize()
    return nc


_NC_CACHE = {}


def _get_nc(rows=ROWS):
    if rows not in _NC_CACHE:
        _NC_CACHE[rows] = build_bass(rows=rows)
    return _NC_CACHE[rows]


def _pack_input(x):
    """x [ROWS, F] f32 -> flat fp16: per-slab [P, 4, g*P] packs,
    pack[p, k, b] = x[slab_row0 + b, 128*k + p]."""
    h = x.astype(np.float16)
    parts = []
    r0 = 0
    for g in SCHED:
        gb = g * P
        blk = h[r0:r0 + gb].reshape(gb, 4, P).transpose(2, 1, 0)
        parts.append(np.ascontiguousarray(blk).reshape(-1))
        r0 += gb
    return np.concatenate(parts)


def run(input_features, pre_w, pre_b, q_weights, post_w, post_b, **spmd_kwargs):
    x = np.asarray(input_features, dtype=np.float32)
    assert x.shape == (B_TOTAL, F_IN), x.shape
    consts = _host_consts(pre_w, pre_b, q_weights, post_w, post_b)
    in_maps = []
    for c in range(N_CORES):
        ht = _pack_input(x[c * ROWS:(c + 1) * ROWS])
        in_maps.append(dict(consts, htp=ht))
    nc = _get_nc()
    r = run_bass_kernel_spmd(nc, in_maps, core_ids=list(range(N_CORES)), **spmd_kwargs)
    # out_dev[t, c, s, p] -> out[(s*G + t)*128 + p, c]
    outs = []
    for c in range(N_CORES):
        o = r.results[c]["out"]                             # [t, c, s, p]
        o = o.transpose(2, 0, 3, 1).reshape(ROWS, 2)        # [s, t, p, c]
        outs.append(o)
    out = np.concatenate(outs, axis=0)
    return out.astype(np.float32), r


def kernel(input_features, pre_w, pre_b, q_weights, post_w, post_b):
    out, _ = run(input_features, pre_w, pre_b, q_weights, post_w, post_b)
    return out


# revision 17
# speedup vs baseline: 1.0671x; 1.0450x over previous
"""DressedQuantumNet on 8 TRN2 NeuronCores (pure data parallel).

Math: pre-net angles th = X @ pre_w.T (+ pre_b).  After H + RY(th') the
4-qubit state is the REAL product state

  psi = kron_w [cos(th'_w/2 + pi/4), sin(th'_w/2 + pi/4)],  th' = th + pre_b

and the rest of the circuit is a FIXED unitary V (depends only on
q_weights).  The per-wire bias rotation R(pre_b_w/2) is absorbed into
V' = V @ kron_w R(pre_b_w/2), so the device only needs raw th.  With
Wz[i,c] = sum_w post_w[c,w] z_w(i) + post_b[c] (sum_i probs_i == 1), the
whole head collapses to two real symmetric quadratic forms

  out_c = psi^T K_c psi,   K_c = Re(V'^H diag(Wz[:,c]) V')   [16 x 16]

Device pipeline per 8-row-tile slab (batch on SBUF partitions), software
pipelined A(s) | B(s-1) | C(s-2) to keep all engine FIFOs unblocked:

  A: DMA fp16 X^T slab (1 MiB, sync queue; consts preloaded on same queue)
     PE  4 accumulating matmuls/tile -> th in PSUM [128,t,4]
     ACT cs = Sin(+-0.5*th + pi/4)  -> (cos,sin) fp16
     DVE psi = (c0,s0)x(c1,s1)x(c2,s2)x(c3,s3)  [128, t, 16] fp16
  B: PE  transpose psi -> psiT [16t, 128] PSUM; ACT copy -> SBUF
     PE  qq_c = blockdiag(K_c) @ psiT  (c=0,1 -> one PSUM tile)
  C: DVE pq_c = psiT * qq_c  (SBUF x PSUM -> fp16)
     PE  z[t, c, p] = column-sum over the 16 states (selection matmul)
     ACT copy z -> resall; one DMA out at the end

Everything sits under the fp16 input-stream DMA floor (~8.4 MiB/core).
"""

from contextlib import ExitStack

import numpy as np

import concourse.bass as bass
import concourse.bacc as bacc_mod
import concourse.mybir as mybir
from concourse.bass_utils import run_bass_kernel_spmd
from concourse.tile import TileContext

N_CORES = 8
B_TOTAL = 65536
F_IN = 512
ROWS = B_TOTAL // N_CORES   # 8192 rows per core
P = 128
N_TILES = ROWS // P         # 64 row-tiles
G = 8                       # row-tiles per slab (1 MiB fp16 DMA)
N_SLABS = N_TILES // G      # 8

F32 = mybir.dt.float32
FP16 = mybir.dt.float16
PI = float(np.pi)

N_QUBITS, VAR_DEPTH = 4, 3


# ----------------------------------------------------------------- host math
def _gate_1q(g, w):
    ops = [np.eye(2, dtype=complex)] * N_QUBITS
    ops[w] = g
    U = ops[0]
    for i in range(1, N_QUBITS):
        U = np.kron(U, ops[i])
    return U


def _bit(i, w):  # wire 0 = most significant
    return (i >> (N_QUBITS - 1 - w)) & 1


def _cnot(c, t):
    M = np.zeros((16, 16), dtype=complex)
    for i in range(16):
        j = i ^ (1 << (N_QUBITS - 1 - t)) if _bit(i, c) else i
        M[j, i] = 1.0
    return M


def _ry(theta):
    c, s = np.cos(theta / 2), np.sin(theta / 2)
    return np.array([[c, -s], [s, c]], dtype=complex)


def _rz(theta):
    ph = np.exp(1j * theta / 2)
    return np.array([[np.conj(ph), 0], [0, ph]], dtype=complex)


def _fixed_unitary(qw):
    V = np.eye(16, dtype=complex)

    def app(Gm):
        nonlocal V
        V = Gm @ V

    def entangle():
        app(_cnot(0, 1)); app(_cnot(2, 3)); app(_cnot(1, 2))

    for k in range(VAR_DEPTH):
        entangle()
        for w in range(N_QUBITS):
            app(_gate_1q(_ry(qw[k, w]), w))
        for w in range(N_QUBITS):
            app(_gate_1q(_rz(qw[k, w]), w))
    for k in range(VAR_DEPTH):
        entangle()
        for w in range(N_QUBITS):
            app(_gate_1q(_ry(qw[k, w]), w))
        for w in range(N_QUBITS):
            app(_gate_1q(_rz(qw[3 + k, w]), w))
    entangle()
    return V


def _host_consts(pre_w, pre_b, q_weights, post_w, post_b):
    pre_w = np.asarray(pre_w, dtype=np.float64)
    pre_b = np.asarray(pre_b, dtype=np.float64)
    post_w = np.asarray(post_w, dtype=np.float64)
    post_b = np.asarray(post_b, dtype=np.float64)

    # whl[p, 4k + w] = pre_w[w, 128k + p]
    whl = np.zeros((P, 16), dtype=np.float16)
    for k in range(4):
        whl[:, 4 * k:4 * k + 4] = pre_w.T[P * k:P * (k + 1)].astype(np.float16)

    V = _fixed_unitary(np.asarray(q_weights, dtype=np.float64))
    R = np.eye(1)
    for w in range(N_QUBITS):
        d = pre_b[w] / 2.0
        R = np.kron(R, np.array([[np.cos(d), -np.sin(d)],
                                 [np.sin(d), np.cos(d)]]))
    Vp = V @ R

    # Wz[i, c] = sum_w post_w[c,w] z_w(i) + post_b[c]  (sum_i probs_i == 1)
    Wz = np.zeros((16, 2))
    for c in range(2):
        for i in range(16):
            Wz[i, c] = sum(
                post_w[c, w] * (1.0 - 2.0 * _bit(i, w)) for w in range(N_QUBITS)
            ) + post_b[c]

    # K_c = Re(V'^H diag(Wz_c) V')  -- real symmetric 16x16; block-diagonal
    kb = []
    for c in range(2):
        Kc = (Vp.conj().T @ np.diag(Wz[:, c]) @ Vp).real
        blk = np.zeros((P, P), dtype=np.float16)
        for t in range(G):
            blk[16 * t:16 * t + 16, 16 * t:16 * t + 16] = Kc.T.astype(np.float16)
        kb.append(blk)

    selz = np.zeros((P, G), dtype=np.float16)
    for t in range(G):
        selz[16 * t:16 * t + 16, t] = 1.0

    ident = np.eye(P, dtype=np.float16)
    blob = np.concatenate([whl, kb[0], kb[1], selz, ident], axis=1)
    return {"cblob": np.ascontiguousarray(blob)}


# ------------------------------------------------------------- device kernel
SCHED = [8, 16, 16, 16, 4, 4]       # tiles per DMA slab (sum = 64)
UNITS = 8                           # 8-tile epilogue units
BATCHES = [4, 2, 2]                 # units per epilogue batch (sum = 8)


def build_bass(rows=ROWS):
    n_tiles = rows // P
    assert sum(SCHED) == n_tiles
    n_units = n_tiles // G

    nc = bacc_mod.Bacc(None, target_bir_lowering=False)
    # host-packed flat: concatenation of per-slab [P, 4, g*P] fp16 blocks
    ht_d = nc.dram_tensor("htp", [rows * 4 * P], FP16, kind="ExternalInput")
    cb_d = nc.dram_tensor("cblob", [P, 16 + 3 * P + G], FP16, kind="ExternalInput")
    # out_dev[t, c, u, p] = out[(u*G + t)*128 + p, c]; host unscrambles
    out_d = nc.dram_tensor("out", [G, 2, n_units, P], F32, kind="ExternalOutput")

    with TileContext(nc) as tc, ExitStack() as ctx:
        const = ctx.enter_context(tc.tile_pool(name="const", bufs=1))
        cblob = const.tile([P, 16 + 3 * P + G], FP16)
        nc.sync.dma_start(cblob, cb_d[:])
        whl = cblob[:, 0:16]
        k0b = cblob[:, 16:16 + P]
        k1b = cblob[:, 16 + P:16 + 2 * P]
        selz = cblob[:, 16 + 2 * P:16 + 2 * P + G]
        ident = cblob[:, 16 + 2 * P + G:16 + 3 * P + G]
        pi4 = const.tile([P, 1], F32)
        nc.vector.memset(pi4, PI / 4)

        xp = ctx.enter_context(tc.tile_pool(name="xin", bufs=len(SCHED)))
        angp = ctx.enter_context(tc.tile_pool(name="angp", bufs=2, space="PSUM"))
        csp = ctx.enter_context(tc.tile_pool(name="csp", bufs=3))
        pp = ctx.enter_context(tc.tile_pool(name="pp", bufs=3))
        psip = ctx.enter_context(tc.tile_pool(name="psip", bufs=4))
        ptp = ctx.enter_context(tc.tile_pool(name="ptp", bufs=1, space="PSUM"))
        pts = ctx.enter_context(tc.tile_pool(name="pts", bufs=2))
        qqp = ctx.enter_context(tc.tile_pool(name="qqp", bufs=1, space="PSUM"))
        prp = ctx.enter_context(tc.tile_pool(name="prp", bufs=2))
        zp = ctx.enter_context(tc.tile_pool(name="zp", bufs=1, space="PSUM"))
        rp = ctx.enter_context(tc.tile_pool(name="res", bufs=1))

        resall = rp.tile([G, 2, n_units, P], F32)

        def stage_a(s, g, base):
            gb = g * P
            ht = xp.tile([P, 4, gb], FP16, tag="ht")
            nc.sync.dma_start(
                ht,
                ht_d[base:base + P * 4 * gb].rearrange(
                    "(p k b) -> p k b", p=P, k=4),
            )
            # th[p, t, w] in PSUM, fp32
            ang = angp.tile([P, g, 4], F32, tag="ang")
            for t in range(g):
                bs = t * P
                for k in range(4):
                    nc.tensor.matmul(
                        ang[:, t, :],
                        ht[:, k, bs:bs + P],
                        whl[:, 4 * k:4 * k + 4],
                        start=(k == 0), stop=(k == 3),
                    )
            # cs[p, t, w, 0] = cos(th/2 + pi/4) = Sin(-.5*th + pi/4)
            # cs[p, t, w, 1] = sin(th/2 + pi/4) = Sin(+.5*th + pi/4)
            cs = csp.tile([P, g, 4, 2], FP16, tag="cs")
            nc.scalar.activation(
                cs[:, :, :, 0], ang, mybir.ActivationFunctionType.Sin,
                bias=pi4, scale=-0.5,
            )
            nc.scalar.activation(
                cs[:, :, :, 1], ang, mybir.ActivationFunctionType.Sin,
                bias=pi4, scale=0.5,
            )
            # psi = kron of the four (c,s) pairs -> [P, g, 4, 4] fp16
            p01 = pp.tile([P, g, 2, 2], FP16, tag="p01")
            nc.vector.tensor_mul(
                p01,
                cs[:, :, 0, :].unsqueeze(3).broadcast_to([P, g, 2, 2]),
                cs[:, :, 1, :].unsqueeze(2).broadcast_to([P, g, 2, 2]),
            )
            p23 = pp.tile([P, g, 2, 2], FP16, tag="p23")
            nc.vector.tensor_mul(
                p23,
                cs[:, :, 2, :].unsqueeze(3).broadcast_to([P, g, 2, 2]),
                cs[:, :, 3, :].unsqueeze(2).broadcast_to([P, g, 2, 2]),
            )
            psi = psip.tile([P, g, 4, 4], FP16, tag="psi")
            nc.vector.tensor_mul(
                psi,
                p01.rearrange("p g a b -> p g (a b)")
                   .unsqueeze(3).broadcast_to([P, g, 4, 4]),
                p23.rearrange("p g a b -> p g (a b)")
                   .unsqueeze(2).broadcast_to([P, g, 4, 4]),
            )
            return psi

        def epilogue(b, nu, u0):
            """Units u0..u0+nu-1: psiT already in ptile; contract and emit z."""
            ptile = ptiles.pop(b)
            psiTs = pts.tile([P, nu, P], FP16, tag="psiTs")
            nc.scalar.copy(psiTs, ptile)
            rhs = psiTs.rearrange("p u b -> p (u b)")
            qq0 = qqp.tile([P, nu, P], F32, tag="qq0")
            nc.tensor.matmul(qq0.rearrange("p u b -> p (u b)"), k0b, rhs,
                             start=True, stop=True)
            qq1 = qqp.tile([P, nu, P], F32, tag="qq1")
            nc.tensor.matmul(qq1.rearrange("p u b -> p (u b)"), k1b, rhs,
                             start=True, stop=True)
            pq = prp.tile([P, 2, nu, P], FP16, tag="pq")
            nc.vector.tensor_mul(pq[:, 0], qq0, psiTs)
            nc.vector.tensor_mul(pq[:, 1], qq1, psiTs)
            for c, qt in ((0, "z0"), (1, "z1")):
                z_ps = zp.tile([G, nu, P], F32, tag=qt)
                nc.tensor.matmul(
                    z_ps.rearrange("t u b -> t (u b)"), selz,
                    pq[:, c].rearrange("p u b -> p (u b)"),
                    start=True, stop=True,
                )
                nc.scalar.copy(resall[:, c, u0:u0 + nu, :], z_ps)
            nc.gpsimd.dma_start(out_d[:, :, u0:u0 + nu, :],
                                resall[:, :, u0:u0 + nu, :])

        # batch/unit layout
        batch_of_unit, ucol_of_unit, u0_of_batch = {}, {}, {}
        u = 0
        for b, nu in enumerate(BATCHES):
            u0_of_batch[b] = u
            for k in range(nu):
                batch_of_unit[u] = b
                ucol_of_unit[u] = k
                u += 1
        n_units_total = u
        # per-slab: dram base + unit chunks (unit, toff, chunk_start, chunk_len)
        slab_info = []
        tile0, base = 0, 0
        for si, g in enumerate(SCHED):
            chunks = []
            t = tile0
            while t < tile0 + g:
                unit, toff = t // G, t % G
                ln = min(G - toff, tile0 + g - t)
                chunks.append((unit, toff, t - tile0, ln))
                t += ln
            slab_info.append((g, base, chunks))
            tile0 += g
            base += P * 4 * g * P
        units_left = {b: nu for b, nu in enumerate(BATCHES)}

        ptiles = {}

        def transpose_slab(si):
            g, base, chunks = slab_info[si]
            psi = psis.pop(si)
            done_batches = []
            for unit, toff, cs0, ln in chunks:
                b = batch_of_unit[unit]
                if b not in ptiles:
                    ptiles[b] = ptp.tile([P, BATCHES[b], P], FP16, tag="pt",
                                         name="ptile")
                nc.tensor.transpose(
                    ptiles[b][16 * toff:16 * (toff + ln), ucol_of_unit[unit], :],
                    psi[:, cs0:cs0 + ln, :, :].rearrange("p g a b -> p (g a b)"),
                    ident,
                )
                if toff + ln == G:
                    units_left[b] -= 1
                    if units_left[b] == 0:
                        done_batches.append(b)
            return done_batches

        psis = {}
        n_slabs = len(SCHED)
        TL = 2                      # transpose emission lag (slabs)
        epi_q = []
        for s in range(n_slabs + TL):
            while epi_q:
                b = epi_q.pop(0)
                epilogue(b, BATCHES[b], u0_of_batch[b])
            if s < n_slabs:
                g, base, chunks = slab_info[s]
                psis[s] = stage_a(s, g, base)
            if s >= TL:
                epi_q.extend(transpose_slab(s - TL))
        while epi_q:
            b = epi_q.pop(0)
            epilogue(b, BATCHES[b], u0_of_batch[b])

    nc.finalize()
    return nc


_NC_CACHE = {}


def _get_nc(rows=ROWS):
    if rows not in _NC_CACHE:
        _NC_CACHE[rows] = build_bass(rows=rows)
    return _NC_CACHE[rows]


def _pack_input(x):
    """x [ROWS, F] f32 -> flat fp16: per-slab [P, 4, g*P] packs,
    pack[p, k, b] = x[slab_row0 + b, 128*k + p]."""
    h = x.astype(np.float16)
    parts = []
    r0 = 0
    for g in SCHED:
        gb = g * P
        blk = h[r0:r0 + gb].reshape(gb, 4, P).transpose(2, 1, 0)
        parts.append(np.ascontiguousarray(blk).reshape(-1))
        r0 += gb
    return np.concatenate(parts)


def run(input_features, pre_w, pre_b, q_weights, post_w, post_b, **spmd_kwargs):
    x = np.asarray(input_features, dtype=np.float32)
    assert x.shape == (B_TOTAL, F_IN), x.shape
    consts = _host_consts(pre_w, pre_b, q_weights, post_w, post_b)
    in_maps = []
    for c in range(N_CORES):
        ht = _pack_input(x[c * ROWS:(c + 1) * ROWS])
        in_maps.append(dict(consts, htp=ht))
    nc = _get_nc()
    r = run_bass_kernel_spmd(nc, in_maps, core_ids=list(range(N_CORES)), **spmd_kwargs)
    # out_dev[t, c, s, p] -> out[(s*G + t)*128 + p, c]
    outs = []
    for c in range(N_CORES):
        o = r.results[c]["out"]                             # [t, c, s, p]
        o = o.transpose(2, 0, 3, 1).reshape(ROWS, 2)        # [s, t, p, c]
        outs.append(o)
    out = np.concatenate(outs, axis=0)
    return out.astype(np.float32), r


def kernel(input_features, pre_w, pre_b, q_weights, post_w, post_b):
    out, _ = run(input_features, pre_w, pre_b, q_weights, post_w, post_b)
    return out
